# revision 8
# baseline (speedup 1.0000x reference)
"""Segment-mean (graph pooling) kernel for Trainium2, 8 NeuronCores.

reference semantics:
    sums   = segment_sum(node_h, node_batch, num_segments=G)
    counts = segment_sum(ones(N), node_batch, G)
    out    = sums / max(counts, 1)[:, None]

node_batch is sorted, so segments are contiguous row runs. Core c owns
segments [128c, 128(c+1)) and reads exactly those segments' rows, cast
to bf16 on the host (input quantization ~2e-3 rel err, well under the
2e-2 gate) and zero-padded per segment to a multiple of 128 rows. With
that padding every 128-row tile belongs to exactly one segment, so the
device only needs per-tile column sums plus a tiny tile->segment
routing matmul:

  stage 1: per tile t, tilesum_t = ones(128) @ rows_t via PE matmuls
           whose lhsT is a shifted one-hot-column "staircase" slice
           (M=32 so out.base_partition stays in {0,32,64,96}), so 32
           consecutive tiles accumulate into one PSUM [32,128] block;
           a chunk of 128 tiles fills a PSUM [128,128] tile.
  stage 2: cast chunk tilesums to bf16 (DVE) and matmul with a
           DVE-built one-hot [tile, seg] selector, accumulating
           [128 segs, 128 feat] in PSUM across all chunks.

Epilogue scales by 1/max(count,1). DMA is the bottleneck by design:
~67 MB/core of bf16 at the ~358 GB/s HBM/NC limit, while PE does one
128-col matmul per tile and DVE does almost nothing.
"""

import os

import numpy as np
import ml_dtypes

BF16 = ml_dtypes.bfloat16
P = 128  # partitions / rows per tile
D = 128  # feature dim
G = 1024  # num segments
N_CORES = 8
SLAB = 64  # node-tiles per DMA slab (2 MiB per slab)
CHUNK = 128  # tiles per PSUM chunk (= 2 slabs)
SLAB_BUFS = 8
SENTINEL = 200.0  # tileseg id outside [0,128) -> routed nowhere

_prog_cache: dict[int, object] = {}
LAST_RESULT = None  # BassKernelResults of the most recent device run


def _np_fallback(node_h, node_batch, num_graphs):
    node_h = np.asarray(node_h, dtype=np.float32)
    nb = np.asarray(node_batch).astype(np.int64)
    ng = int(num_graphs)
    sums = np.zeros((ng, node_h.shape[1]), dtype=np.float32)
    np.add.at(sums, nb, node_h)
    counts = np.bincount(nb, minlength=ng).astype(np.float32)
    return sums / np.maximum(counts, 1.0)[:, None]


def _build_program(T: int):
    import concourse.bacc as bacc
    import concourse.mybir as mybir
    import concourse.tile as tile

    bf16 = mybir.dt.bfloat16
    f32 = mybir.dt.float32

    assert T % CHUNK == 0
    n_chunks = T // CHUNK
    n_slabs = T // SLAB
    META_W = P + 127 + n_chunks  # iota | staircase | tileseg

    nc = bacc.Bacc(None)
    h_in = nc.dram_tensor("h", [P, T * D], bf16, kind="ExternalInput")
    meta_in = nc.dram_tensor("meta", [P, META_W], bf16, kind="ExternalInput")
    recip_in = nc.dram_tensor("recip", [P, 1], f32, kind="ExternalInput")
    out_t = nc.dram_tensor("out", [P, D], f32, kind="ExternalOutput")

    with tile.TileContext(nc) as tc:
        with (
            tc.tile_pool(name="const", bufs=1) as constp,
            tc.tile_pool(name="slabs", bufs=SLAB_BUFS) as slabp,
            tc.tile_pool(name="ts", bufs=2) as tsp,
            tc.tile_pool(name="oh", bufs=2) as ohp,
            tc.tile_pool(name="chunk", bufs=3, space="PSUM") as chunkp,
            tc.tile_pool(name="acc", bufs=1, space="PSUM") as accp,
            tc.tile_pool(name="outp", bufs=1) as outp,
        ):
            meta_sb = constp.tile([P, META_W], bf16)
            nc.sync.dma_start(meta_sb[:], meta_in[:])
            recip_sb = constp.tile([P, 1], f32)
            nc.sync.dma_start(recip_sb[:], recip_in[:])
            iota = meta_sb[:, 0:P]
            stair = meta_sb[:, P : P + 127]
            tileseg0 = P + 127

            acc = accp.tile([P, D], f32)

            slabs = {}

            for k in range(n_chunks):
                # tile->segment one-hot for this chunk's 128 tiles
                oh = ohp.tile([P, P], bf16)
                nc.vector.tensor_tensor(
                    out=oh[:],
                    in0=iota,
                    in1=meta_sb[:, tileseg0 + k : tileseg0 + k + 1].to_broadcast(
                        [P, P]
                    ),
                    op=mybir.AluOpType.is_equal,
                )

                # stage 1: column sums of 128 tiles into PSUM rows
                cp = chunkp.tile([P, D], f32)
                for a in range(2):
                    for j in range(64):
                        t = k * CHUNK + a * 64 + j
                        g = t // SLAB
                        if t % SLAB == 0:
                            slabs[g] = slabp.tile(
                                [P, SLAB * D], bf16, name="slab"
                            )
                            nc.sync.dma_start(
                                slabs[g][:],
                                h_in[:, g * SLAB * D : (g + 1) * SLAB * D],
                            )
                        pos = t % SLAB
                        nc.tensor.matmul(
                            out=cp[a * 64 : (a + 1) * 64, :],
                            lhsT=stair[:, 63 - j : 127 - j],
                            rhs=slabs[g][:, pos * D : (pos + 1) * D],
                            start=(j == 0),
                            stop=(j == 63),
                        )

                # stage 2: route tilesums to segment rows
                ts = tsp.tile([P, D], bf16)
                nc.vector.tensor_copy(out=ts[:], in_=cp[:])
                nc.tensor.matmul(
                    out=acc[:],
                    lhsT=oh[:],
                    rhs=ts[:],
                    start=(k == 0),
                    stop=(k == n_chunks - 1),
                    skip_group_check=True,
                )

            res = outp.tile([P, D], f32)
            nc.vector.tensor_tensor(
                out=res[:],
                in0=acc[:],
                in1=recip_sb[:, 0:1].to_broadcast([P, D]),
                op=mybir.AluOpType.mult,
            )
            nc.sync.dma_start(out_t[:], res[:])

    nc.finalize()
    return nc


def kernel(node_h, node_batch, num_graphs):
    global LAST_RESULT
    node_h = np.asarray(node_h)
    nb = np.asarray(node_batch)
    ng = int(num_graphs)

    N = node_h.shape[0]
    if (
        ng != G
        or node_h.ndim != 2
        or node_h.shape[1] != D
        or nb.shape != (N,)
        or np.any(nb[:-1] > nb[1:])
        or nb[0] < 0
        or nb[-1] >= G
    ):
        return _np_fallback(node_h, node_batch, num_graphs)

    node_h = np.ascontiguousarray(node_h, dtype=np.float32)
    nb = nb.astype(np.int64)

    seg_per_core = G // N_CORES
    counts = np.bincount(nb, minlength=G)
    bounds = np.concatenate([[0], np.cumsum(counts)])
    kt = -(-counts // P)  # tiles per segment after zero padding
    per_core_tiles = kt.reshape(N_CORES, seg_per_core).sum(axis=1)
    T = int(-(-per_core_tiles.max() // CHUNK) * CHUNK)
    if T < 2 * SLAB or T > 4096:
        return _np_fallback(node_h, node_batch, num_graphs)
    n_chunks = T // CHUNK

    iota = np.tile(np.arange(P, dtype=np.float32), (P, 1))
    stair = np.zeros((P, 127), dtype=np.float32)
    stair[:, 63] = 1.0

    in_maps = []
    for c in range(N_CORES):
        s0 = c * seg_per_core
        r0, r1 = int(bounds[s0]), int(bounds[s0 + seg_per_core])
        blk = node_h[r0:r1].astype(BF16)

        h = np.zeros((P, T * D), dtype=BF16)
        ts_flat = np.full(T, SENTINEL, dtype=np.float32)
        off = 0
        for i in range(seg_per_core):
            s = s0 + i
            cnt = int(counts[s])
            if cnt == 0:
                continue
            k = int(kt[s])
            stage = np.zeros((P * k, D), dtype=BF16)
            stage[:cnt] = blk[bounds[s] - r0 : bounds[s + 1] - r0]
            h[:, off * D : (off + k) * D] = stage.reshape(P, k * D)
            ts_flat[off : off + k] = i
            off += k

        tileseg = np.ascontiguousarray(ts_flat.reshape(n_chunks, P).T)
        meta = np.concatenate([iota, stair, tileseg], axis=1).astype(BF16)
        recip = (
            1.0
            / np.maximum(counts[s0 : s0 + seg_per_core], 1.0).astype(np.float32)
        ).reshape(P, 1)

        in_maps.append({"h": h, "meta": meta, "recip": recip})

    if T not in _prog_cache:
        _prog_cache[T] = _build_program(T)
    nc = _prog_cache[T]

    from concourse.bass_utils import run_bass_kernel_spmd

    trace = bool(os.environ.get("KERNEL_TRACE"))
    result = run_bass_kernel_spmd(
        nc,
        in_maps,
        core_ids=list(range(N_CORES)),
        trace=trace,
        trace_cores=list(range(N_CORES)) if trace else None,
    )
    LAST_RESULT = result

    out = np.concatenate([result.results[c]["out"] for c in range(N_CORES)], axis=0)
    return out.astype(np.float32)


# revision 10
# speedup vs baseline: 1.0345x; 1.0345x over previous
"""Segment-mean (graph pooling) kernel for Trainium2, 8 NeuronCores.

reference semantics:
    sums   = segment_sum(node_h, node_batch, num_segments=G)
    counts = segment_sum(ones(N), node_batch, G)
    out    = sums / max(counts, 1)[:, None]

node_batch is sorted, so segments are contiguous row runs. Core c owns
segments [128c, 128(c+1)) and reads exactly those segments' rows, cast
to bf16 on the host (input quantization ~2e-3 rel err, well under the
2e-2 gate) and zero-padded per segment to a multiple of 128 rows. With
that padding every 128-row tile belongs to exactly one segment, so the
device only needs per-tile column sums plus a tiny tile->segment
routing matmul:

  stage 1: tilesum_t = ones(128) @ rows_t. One matmul covers FOUR
           adjacent tiles (rhs [128, 512] = a full PSUM bank row);
           its lhsT is a shifted one-hot-column "staircase" slice
           [128, 32] so MM q of a chunk lands its 4 tilesums on PSUM
           partition q. 32 accumulating MMs = one chunk of 128 tiles
           -> PSUM [32, 512] where [q, j*128:...] = tilesum(4q+j).
  stage 2: cast chunk tilesums to bf16 (DVE) and 4 matmuls/chunk with
           a DVE-built one-hot [tile, seg] selector (K=32), all
           accumulating [128 segs, 128 feat] in PSUM across chunks.

Epilogue scales by 1/max(count,1). DMA is the bottleneck by design:
~67 MB/core of bf16 at the HBM/NC limit; PE streams the same columns
as the DMA delivers (~120us) and DVE does almost nothing. Keeping the
instruction count low (~650, one 64B instruction per 4-tile MM) keeps
the Q14 instruction-refill DMAs off engine 0's port, which otherwise
becomes the straggler DMA engine.
"""

import os

import numpy as np
import ml_dtypes

BF16 = ml_dtypes.bfloat16
P = 128  # partitions / rows per tile
D = 128  # feature dim
G = 1024  # num segments
N_CORES = 8
SLAB = 64  # node-tiles per DMA slab (2 MiB per slab)
CHUNK = 128  # tiles per PSUM chunk (= 2 slabs, 32 four-tile MMs)
SLAB_BUFS = 8
SENTINEL = 200.0  # tileseg id outside [0,128) -> routed nowhere

_prog_cache: dict[int, object] = {}
LAST_RESULT = None  # BassKernelResults of the most recent device run


def _np_fallback(node_h, node_batch, num_graphs):
    node_h = np.asarray(node_h, dtype=np.float32)
    nb = np.asarray(node_batch).astype(np.int64)
    ng = int(num_graphs)
    sums = np.zeros((ng, node_h.shape[1]), dtype=np.float32)
    np.add.at(sums, nb, node_h)
    counts = np.bincount(nb, minlength=ng).astype(np.float32)
    return sums / np.maximum(counts, 1.0)[:, None]


def _build_program(T: int):
    import concourse.bacc as bacc
    import concourse.mybir as mybir
    import concourse.tile as tile

    bf16 = mybir.dt.bfloat16
    f32 = mybir.dt.float32

    assert T % CHUNK == 0
    n_chunks = T // CHUNK
    META_W = P + 63 + T // 32  # iota | staircase | tileseg

    nc = bacc.Bacc(None)
    h_in = nc.dram_tensor("h", [P, T * D], bf16, kind="ExternalInput")
    meta_in = nc.dram_tensor("meta", [P, META_W], bf16, kind="ExternalInput")
    recip_in = nc.dram_tensor("recip", [P, 1], f32, kind="ExternalInput")
    out_t = nc.dram_tensor("out", [P, D], f32, kind="ExternalOutput")

    with tile.TileContext(nc) as tc:
        with (
            tc.tile_pool(name="const", bufs=1) as constp,
            tc.tile_pool(name="slabs", bufs=SLAB_BUFS) as slabp,
            tc.tile_pool(name="ts", bufs=2) as tsp,
            tc.tile_pool(name="oh", bufs=2) as ohp,
            tc.tile_pool(name="chunk", bufs=2, space="PSUM") as chunkp,
            tc.tile_pool(name="acc", bufs=1, space="PSUM") as accp,
            tc.tile_pool(name="outp", bufs=1) as outp,
        ):
            slabs = {}

            def slab_dma(g):
                slabs[g] = slabp.tile([P, SLAB * D], bf16, name="slab")
                nc.sync.dma_start(
                    slabs[g][:], h_in[:, g * SLAB * D : (g + 1) * SLAB * D]
                )

            # first slab ahead of the metadata so the bulk stream starts
            # immediately; meta/recip are tiny and slot in behind it
            slab_dma(0)
            meta_sb = constp.tile([P, META_W], bf16)
            nc.sync.dma_start(meta_sb[:], meta_in[:])
            recip_sb = constp.tile([P, 1], f32)
            nc.sync.dma_start(recip_sb[:], recip_in[:])
            iota = meta_sb[:, 0:P]
            stair = meta_sb[:, P : P + 63]
            tileseg0 = P + 63

            acc = accp.tile([P, D], f32)

            for k in range(n_chunks):
                # tile->segment one-hot for this chunk's 128 tiles:
                # oh[q, j, s] = (seg(tile 128k+4q+j) == s)
                oh = ohp.tile([32, 4 * P], bf16)
                nc.vector.tensor_tensor(
                    out=oh[:].rearrange("p (a b) -> p a b", b=P),
                    in0=iota[0:32, :].unsqueeze(1).to_broadcast([32, 4, P]),
                    in1=meta_sb[0:32, tileseg0 + 4 * k : tileseg0 + 4 * k + 4]
                    .unsqueeze(2)
                    .to_broadcast([32, 4, P]),
                    op=mybir.AluOpType.is_equal,
                )

                # stage 1: 32 four-tile column-sum MMs -> PSUM [32, 512]
                cp = chunkp.tile([32, 4 * D], f32)
                for q in range(32):
                    t = k * CHUNK + 4 * q
                    g = t // SLAB
                    if t % SLAB == 0 and g not in slabs:
                        slab_dma(g)
                    pos = t % SLAB
                    nc.tensor.matmul(
                        out=cp[:],
                        lhsT=stair[:, 31 - q : 63 - q],
                        rhs=slabs[g][:, pos * D : (pos + 4) * D],
                        start=(q == 0),
                        stop=(q == 31),
                    )

                # stage 2: route tilesums to segment rows
                ts = tsp.tile([32, 4 * D], bf16)
                nc.vector.tensor_copy(out=ts[:], in_=cp[:])
                for j in range(4):
                    nc.tensor.matmul(
                        out=acc[:],
                        lhsT=oh[:, j * P : (j + 1) * P],
                        rhs=ts[:, j * D : (j + 1) * D],
                        start=(k == 0 and j == 0),
                        stop=(k == n_chunks - 1 and j == 3),
                        skip_group_check=True,
                    )

            res = outp.tile([P, D], f32)
            nc.vector.tensor_tensor(
                out=res[:],
                in0=acc[:],
                in1=recip_sb[:, 0:1].to_broadcast([P, D]),
                op=mybir.AluOpType.mult,
            )
            nc.sync.dma_start(out_t[:], res[:])

    nc.finalize()
    return nc


def kernel(node_h, node_batch, num_graphs):
    global LAST_RESULT
    node_h = np.asarray(node_h)
    nb = np.asarray(node_batch)
    ng = int(num_graphs)

    N = node_h.shape[0]
    if (
        ng != G
        or node_h.ndim != 2
        or node_h.shape[1] != D
        or nb.shape != (N,)
        or np.any(nb[:-1] > nb[1:])
        or nb[0] < 0
        or nb[-1] >= G
    ):
        return _np_fallback(node_h, node_batch, num_graphs)

    node_h = np.ascontiguousarray(node_h, dtype=np.float32)
    nb = nb.astype(np.int64)

    seg_per_core = G // N_CORES
    counts = np.bincount(nb, minlength=G)
    bounds = np.concatenate([[0], np.cumsum(counts)])
    kt = -(-counts // P)  # tiles per segment after zero padding
    per_core_tiles = kt.reshape(N_CORES, seg_per_core).sum(axis=1)
    T = int(-(-per_core_tiles.max() // CHUNK) * CHUNK)
    if T < 2 * SLAB or T > 4096:
        return _np_fallback(node_h, node_batch, num_graphs)
    n_chunks = T // CHUNK

    iota = np.tile(np.arange(P, dtype=np.float32), (P, 1))
    stair = np.zeros((P, 63), dtype=np.float32)
    stair[:, 31] = 1.0

    in_maps = []
    for c in range(N_CORES):
        s0 = c * seg_per_core
        r0, r1 = int(bounds[s0]), int(bounds[s0 + seg_per_core])
        blk = node_h[r0:r1].astype(BF16)

        h = np.zeros((P, T * D), dtype=BF16)
        ts_flat = np.full(T, SENTINEL, dtype=np.float32)
        off = 0
        for i in range(seg_per_core):
            s = s0 + i
            cnt = int(counts[s])
            if cnt == 0:
                continue
            k = int(kt[s])
            stage = np.zeros((P * k, D), dtype=BF16)
            stage[:cnt] = blk[bounds[s] - r0 : bounds[s + 1] - r0]
            h[:, off * D : (off + k) * D] = stage.reshape(P, k * D)
            ts_flat[off : off + k] = i
            off += k

        # tileseg[q, 4k+j] = local seg of tile 128k + 4q + j
        tileseg32 = ts_flat.reshape(n_chunks, 32, 4).transpose(1, 0, 2).reshape(
            32, n_chunks * 4
        )
        tileseg = np.zeros((P, n_chunks * 4), dtype=np.float32)
        tileseg[0:32] = tileseg32
        meta = np.concatenate([iota, stair, tileseg], axis=1).astype(BF16)
        recip = (
            1.0
            / np.maximum(counts[s0 : s0 + seg_per_core], 1.0).astype(np.float32)
        ).reshape(P, 1)

        in_maps.append({"h": h, "meta": meta, "recip": recip})

    if T not in _prog_cache:
        _prog_cache[T] = _build_program(T)
    nc = _prog_cache[T]

    from concourse.bass_utils import run_bass_kernel_spmd

    trace = bool(os.environ.get("KERNEL_TRACE"))
    result = run_bass_kernel_spmd(
        nc,
        in_maps,
        core_ids=list(range(N_CORES)),
        trace=trace,
        trace_cores=list(range(N_CORES)) if trace else None,
    )
    LAST_RESULT = result

    out = np.concatenate([result.results[c]["out"] for c in range(N_CORES)], axis=0)
    return out.astype(np.float32)


# revision 11
# speedup vs baseline: 1.1452x; 1.1070x over previous
"""Segment-mean (graph pooling) kernel for Trainium2, 8 NeuronCores.

reference semantics:
    sums   = segment_sum(node_h, node_batch, num_segments=G)
    counts = segment_sum(ones(N), node_batch, G)
    out    = sums / max(counts, 1)[:, None]

node_batch is sorted, so segments are contiguous row runs. Core c owns
segments [128c, 128(c+1)) and reads exactly those segments' rows, cast
to bf16 on the host (input quantization ~2e-3 rel err, well under the
2e-2 gate) and zero-padded per segment to a multiple of 32 rows. With
that padding every 32-row QUARTER of a 128-row tile belongs to exactly
one segment, so the device only needs per-quarter column sums plus a
small quarter->segment routing matmul:

  stage 1: one matmul covers FOUR adjacent tiles (rhs [128, 512] = a
           full PSUM bank row); its lhsT [128, 64] is a sliding slice
           of a "staircase" whose four active columns are the quarter
           indicators (ones on rows 32m..32m+31), placed at column
           4w, so MM (h, w) of a 128-tile chunk lands its 16
           quarter-sums on PSUM partitions 64h + 4w + m. 32 MMs =
           one chunk -> PSUM [128, 512] of quarter-sums.
  stage 2: cast chunk quarter-sums to bf16 (DVE) and 4 matmuls/chunk
           (K=128) with a DVE-built one-hot [quarter, seg] selector,
           accumulating [128 segs, 128 feat] in PSUM across chunks.

Epilogue scales by 1/max(count,1). DMA is the bottleneck by design:
~65 MB/core of bf16 at the HBM/NC limit; PE streams the same columns
the DMA delivers (~115us) and DVE does almost nothing. The low
instruction count (~640, one 64B instruction per 4-tile MM) keeps the
Q14 instruction-refill DMAs small so no DMA engine's port is loaded
beyond its share of the data.
"""

import os

import numpy as np
import ml_dtypes

BF16 = ml_dtypes.bfloat16
P = 128  # partitions / rows per tile
D = 128  # feature dim
G = 1024  # num segments
N_CORES = 8
SLAB = 64  # node-tiles per DMA slab (2 MiB per slab)
CHUNK = 128  # tiles per PSUM chunk (= 2 slabs, 32 four-tile MMs)
SLAB_BUFS = 8
QPAD = 32  # segments padded to a multiple of this many rows
SENTINEL = 200.0  # quarter seg id outside [0,128) -> routed nowhere

_prog_cache: dict[int, object] = {}
LAST_RESULT = None  # BassKernelResults of the most recent device run


def _np_fallback(node_h, node_batch, num_graphs):
    node_h = np.asarray(node_h, dtype=np.float32)
    nb = np.asarray(node_batch).astype(np.int64)
    ng = int(num_graphs)
    sums = np.zeros((ng, node_h.shape[1]), dtype=np.float32)
    np.add.at(sums, nb, node_h)
    counts = np.bincount(nb, minlength=ng).astype(np.float32)
    return sums / np.maximum(counts, 1.0)[:, None]


def _build_program(T: int):
    import concourse.bacc as bacc
    import concourse.mybir as mybir
    import concourse.tile as tile

    bf16 = mybir.dt.bfloat16
    f32 = mybir.dt.float32

    assert T % SLAB == 0
    n_groups = -(-T // CHUNK)  # chunks incl. a possible 64-tile tail
    tail64 = (T % CHUNK) == SLAB
    u_cnt = n_groups
    META_W = P + 124 + u_cnt * 4  # iota | staircase | qseg

    nc = bacc.Bacc(None)
    h_in = nc.dram_tensor("h", [P, T * D], bf16, kind="ExternalInput")
    meta_in = nc.dram_tensor("meta", [P, META_W], bf16, kind="ExternalInput")
    recip_in = nc.dram_tensor("recip", [P, 1], f32, kind="ExternalInput")
    out_t = nc.dram_tensor("out", [P, D], f32, kind="ExternalOutput")

    with tile.TileContext(nc) as tc:
        with (
            tc.tile_pool(name="const", bufs=1) as constp,
            tc.tile_pool(name="slabs", bufs=SLAB_BUFS) as slabp,
            tc.tile_pool(name="ts", bufs=2) as tsp,
            tc.tile_pool(name="oh", bufs=2) as ohp,
            tc.tile_pool(name="chunk", bufs=2, space="PSUM") as chunkp,
            tc.tile_pool(name="acc", bufs=1, space="PSUM") as accp,
            tc.tile_pool(name="outp", bufs=1) as outp,
        ):
            slabs = {}

            def slab_dma(g):
                slabs[g] = slabp.tile([P, SLAB * D], bf16, name="slab")
                nc.sync.dma_start(
                    slabs[g][:], h_in[:, g * SLAB * D : (g + 1) * SLAB * D]
                )

            # first slab ahead of the metadata so the bulk stream starts
            # immediately; meta/recip are tiny and slot in behind it
            slab_dma(0)
            meta_sb = constp.tile([P, META_W], bf16)
            nc.sync.dma_start(meta_sb[:], meta_in[:])
            recip_sb = constp.tile([P, 1], f32)
            nc.sync.dma_start(recip_sb[:], recip_in[:])
            iota = meta_sb[:, 0:P]
            stair = meta_sb[:, P : P + 124]
            qseg0 = P + 124

            acc = accp.tile([P, D], f32)

            for u in range(n_groups):
                is_tail = tail64 and u == n_groups - 1
                mrows = 64 if is_tail else 128
                halves = 1 if is_tail else 2

                # quarter->segment one-hot for this chunk:
                # oh[64h+4w+m, j, s] = (seg(quarter m of tile 128u+64h+4w+j) == s)
                oh = ohp.tile([P, 4 * P], bf16, name="oh")
                nc.vector.tensor_tensor(
                    out=oh[0:mrows, :].rearrange("p (a b) -> p a b", b=P),
                    in0=iota[0:mrows, :].unsqueeze(1).to_broadcast([mrows, 4, P]),
                    in1=meta_sb[0:mrows, qseg0 + 4 * u : qseg0 + 4 * u + 4]
                    .unsqueeze(2)
                    .to_broadcast([mrows, 4, P]),
                    op=mybir.AluOpType.is_equal,
                )

                # stage 1: 16 four-tile quarter-sum MMs per half-chunk
                cp = chunkp.tile([P, 4 * D], f32, name="cp")
                for h in range(halves):
                    g = 2 * u + h
                    slab_dma(g)
                    for w in range(16):
                        pos = 4 * w
                        nc.tensor.matmul(
                            out=cp[h * 64 : h * 64 + 64, :],
                            lhsT=stair[:, 60 - 4 * w : 124 - 4 * w],
                            rhs=slabs[g][:, pos * D : (pos + 4) * D],
                            start=(w == 0),
                            stop=(w == 15),
                        )

                # stage 2: route quarter-sums to segment rows
                ts = tsp.tile([P, 4 * D], bf16, name="ts")
                nc.vector.tensor_copy(out=ts[0:mrows, :], in_=cp[0:mrows, :])
                for j in range(4):
                    nc.tensor.matmul(
                        out=acc[:],
                        lhsT=oh[0:mrows, j * P : (j + 1) * P],
                        rhs=ts[0:mrows, j * D : (j + 1) * D],
                        start=(u == 0 and j == 0),
                        stop=(u == n_groups - 1 and j == 3),
                        skip_group_check=True,
                    )

            res = outp.tile([P, D], f32)
            nc.vector.tensor_tensor(
                out=res[:],
                in0=acc[:],
                in1=recip_sb[:, 0:1].to_broadcast([P, D]),
                op=mybir.AluOpType.mult,
            )
            nc.sync.dma_start(out_t[:], res[:])

    nc.finalize()
    return nc


def kernel(node_h, node_batch, num_graphs):
    global LAST_RESULT
    node_h = np.asarray(node_h)
    nb = np.asarray(node_batch)
    ng = int(num_graphs)

    N = node_h.shape[0]
    if (
        ng != G
        or node_h.ndim != 2
        or node_h.shape[1] != D
        or nb.shape != (N,)
        or np.any(nb[:-1] > nb[1:])
        or nb[0] < 0
        or nb[-1] >= G
    ):
        return _np_fallback(node_h, node_batch, num_graphs)

    node_h = np.ascontiguousarray(node_h, dtype=np.float32)
    nb = nb.astype(np.int64)

    seg_per_core = G // N_CORES
    counts = np.bincount(nb, minlength=G)
    bounds = np.concatenate([[0], np.cumsum(counts)])
    pad_rows = (-counts) % QPAD
    per_core_rows = (counts + pad_rows).reshape(N_CORES, seg_per_core).sum(axis=1)
    T = int(-(-(-(-per_core_rows.max() // P)) // SLAB) * SLAB)
    if T < 2 * SLAB or T > 4096:
        return _np_fallback(node_h, node_batch, num_graphs)
    u_cnt = -(-T // CHUNK)

    iota = np.tile(np.arange(P, dtype=np.float32), (P, 1))
    # staircase: four quarter-indicator columns at positions 60..63
    stair = np.zeros((P, 124), dtype=np.float32)
    for m in range(4):
        stair[32 * m : 32 * (m + 1), 60 + m] = 1.0

    in_maps = []
    for c in range(N_CORES):
        s0 = c * seg_per_core
        r0, r1 = int(bounds[s0]), int(bounds[s0 + seg_per_core])
        blk = node_h[r0:r1].astype(BF16)

        vrows = np.zeros((T * P, D), dtype=BF16)
        qseg_flat = np.full(u_cnt * CHUNK * 4, SENTINEL, dtype=np.float32)
        off = 0
        for i in range(seg_per_core):
            s = s0 + i
            cnt = int(counts[s])
            if cnt == 0:
                continue
            kq = cnt + int(pad_rows[s])
            vrows[off : off + cnt] = blk[bounds[s] - r0 : bounds[s + 1] - r0]
            qseg_flat[off // QPAD : (off + kq) // QPAD] = i
            off += kq

        h = np.ascontiguousarray(
            vrows.reshape(T, P, D).transpose(1, 0, 2)
        ).reshape(P, T * D)
        # qseg[64h+4w+m, 4u+j] = local seg of quarter m of tile 128u+64h+4w+j
        A = qseg_flat.reshape(u_cnt, 2, 16, 4, 4)
        qseg = np.ascontiguousarray(A.transpose(1, 2, 4, 0, 3)).reshape(
            P, u_cnt * 4
        )
        meta = np.concatenate([iota, stair, qseg], axis=1).astype(BF16)
        recip = (
            1.0
            / np.maximum(counts[s0 : s0 + seg_per_core], 1.0).astype(np.float32)
        ).reshape(P, 1)

        in_maps.append({"h": h, "meta": meta, "recip": recip})

    if T not in _prog_cache:
        _prog_cache[T] = _build_program(T)
    nc = _prog_cache[T]

    from concourse.bass_utils import run_bass_kernel_spmd

    trace = bool(os.environ.get("KERNEL_TRACE"))
    result = run_bass_kernel_spmd(
        nc,
        in_maps,
        core_ids=list(range(N_CORES)),
        trace=trace,
        trace_cores=list(range(N_CORES)) if trace else None,
    )
    LAST_RESULT = result

    out = np.concatenate([result.results[c]["out"] for c in range(N_CORES)], axis=0)
    return out.astype(np.float32)
